# revision 14
# baseline (speedup 1.0000x reference)
"""2D DCT-II (unnormalized), 4096x4096, on 8 NeuronCores via Bass/Tile.

Math: Z = C @ X @ C^T with C[k,m] = cos(pi*k*(2m+1)/(2n)), n = 4096.

Even/odd folding on BOTH axes (C[k, n-1-m] = (-1)^k C[k, m]) splits the
transform into four independent half-size ones:

    Z[::2,  ::2] = Ce @ Ass @ Ce^T      Ass = Xtt + Xbt + Xtb + Xbb
    Z[1::2, ::2] = Co @ Ads @ Ce^T      Ads = Xtt - Xbt + Xtb - Xbb
    Z[::2, 1::2] = Ce @ Asd @ Co^T      Asd = Xtt + Xbt - Xtb - Xbb
    Z[1::2,1::2] = Co @ Add @ Co^T      Add = Xtt - Xbt - Xtb + Xbb

where Xtt = X[:h,:h], Xbt = X[h:,:h] row-mirrored, Xtb col-mirrored,
Xbb both, h = 2048, Ce/Co[r, m] = cos(pi*(2r|2r+1)*(2m+1)/(2n)).
The folds + final interleave run on host; the four 2048-transforms run on
the 8 cores (2 cores per quarter, each computing 1024 output rows).

On-device each core runs two matmul passes using the PE primitive
MM(A, B) = A^T @ B (contraction over partitions):

    S1 = MM(A, C1^T[:, chunk])     [2048, 1024]   (stays in SBUF)
    Zq = MM(S1, C2^T)              [1024, 2048]

No transposes, no cross-core communication. Matmuls run in float32r
(PE reads fp32 truncated to ~fp22; full rate for moving dim >= 256).
All DRAM operands are pre-packed on host so every DMA line is contiguous.
"""

import os
import numpy as np

import concourse.bacc as bacc
import concourse.mybir as mybir
import concourse.tile as tile
from concourse.bass_utils import run_bass_kernel_spmd

FULL = 4096
H = 2048                 # half size
P = 128                  # partitions
NCORES = 8
NT = H // P              # 16 tiles of 128 along a 2048 axis
KCH = 1024               # output rows per core (half of a quarter)
F32 = mybir.dt.float32
F32R = mybir.dt.float32r

_cache = {}


def _half_dcts():
    """Ce, Co as [r, m] (float64): rows 2r / 2r+1 of the full DCT matrix."""
    r = np.arange(H, dtype=np.float64)[:, None]
    m = np.arange(H, dtype=np.float64)[None, :]
    ce = np.cos(np.pi * (2 * r) * (2 * m + 1) / (2.0 * FULL))
    co = np.cos(np.pi * (2 * r + 1) * (2 * m + 1) / (2.0 * FULL))
    return ce, co


def _build_nc():
    nc = bacc.Bacc("TRN2", target_bir_lowering=False, debug=False,
                   num_devices=NCORES)
    # a_p[n_t, m_in, m_t, n_in] = A[128*m_t + m_in, 128*n_t + n_in]
    a_p = nc.dram_tensor("a_p", [NT, P, NT, P], F32R,
                         kind="ExternalInput").ap()
    # c1_p[m_in, m_t, k] = C1^T[128*m_t + m_in, KCH*h + k]
    c1_p = nc.dram_tensor("c1_p", [P, NT, KCH], F32R,
                          kind="ExternalInput").ap()
    # c2_p[l_s, n_t, n_in, l] = C2^T[128*n_t + n_in, 512*l_s + l]
    c2_p = nc.dram_tensor("c2_p", [4, NT, P, 512], F32R,
                          kind="ExternalInput").ap()
    z = nc.dram_tensor("z", [KCH, H], F32, kind="ExternalOutput").ap()

    with tile.TileContext(nc) as tc:
        with (
            tc.tile_pool(name="c1", bufs=1) as c1_pool,
            tc.tile_pool(name="s1p", bufs=1) as s1_pool,
            tc.tile_pool(name="ap", bufs=3) as a_pool,
            tc.tile_pool(name="c2", bufs=12) as c2_pool,
            tc.tile_pool(name="out", bufs=4) as out_pool,
            tc.tile_pool(name="ps", bufs=8, space="PSUM") as psum_pool,
        ):
            c1sb = c1_pool.tile([P, NT, KCH], F32R)
            s1 = s1_pool.tile([P, NT, KCH], F32R)

            # PE warmup: accumulate exact zeros into the first psum tiles
            # while the initial DMAs are in flight, so HAM reaches 2.4 GHz
            # before real work arrives (and the real m-loop starts with
            # start=False on pre-zeroed banks).
            zt = c1_pool.tile([P, 512], F32, name="zt")
            nc.gpsimd.memset(zt[:], 0.0)
            ztr = c1_pool.tile([P, 512], F32R, name="ztr")
            nc.vector.tensor_copy(ztr[:], zt[:])
            ps0_first = psum_pool.tile([P, 512], F32, tag="ps", name="p1a_0")
            ps1_first = psum_pool.tile([P, 512], F32, tag="ps", name="p1b_0")
            NWARM = 12
            for w in range(NWARM):
                tgt = ps0_first if w % 2 == 0 else ps1_first
                nc.tensor.matmul(tgt[:], ztr[:, 0:P], ztr[:],
                                 start=(w < 2), stop=False)

            def filler(n, tgt):
                # zero-accumulating matmuls emitted between real ones keep
                # the PE duty cycle high (HAM warm) while the c1 preload is
                # still streaming in.
                for _ in range(n):
                    nc.tensor.matmul(tgt[:], ztr[:, 0:P], ztr[:],
                                     start=False, stop=False)

            # pass 1: S1[:, n_t, :] = sum_m A[m, n_t-block]^T @ C1^T-chunk
            # n_t == 0 interleaves the c1 strip loads in consumption order.
            for n_t in range(NT):
                a_st = a_pool.tile([P, NT, P], F32R, tag="ap",
                                   name=f"a_{n_t}")
                for g in range(4):
                    nc.sync.dma_start(a_st[:, 4 * g:4 * (g + 1), :],
                                      a_p[n_t, :, 4 * g:4 * (g + 1), :])
                if n_t == 0:
                    ps0, ps1 = ps0_first, ps1_first
                else:
                    ps0 = psum_pool.tile([P, 512], F32, tag="ps",
                                         name=f"p1a_{n_t}")
                    ps1 = psum_pool.tile([P, 512], F32, tag="ps",
                                         name=f"p1b_{n_t}")
                for m_t in range(NT):
                    if n_t == 0:
                        for s in range(2):
                            nc.sync.dma_start(
                                c1sb[:, m_t, 512 * s:512 * (s + 1)],
                                c1_p[:, m_t, 512 * s:512 * (s + 1)])
                    if n_t == 0 and m_t > 0:
                        filler(6, ps0)
                    elif n_t == 1:
                        filler(2, ps0)
                    nc.tensor.matmul(ps0[:], a_st[:, m_t, :],
                                     c1sb[:, m_t, 0:512],
                                     start=False if n_t == 0 else (m_t == 0),
                                     stop=(m_t == NT - 1))
                    nc.tensor.matmul(ps1[:], a_st[:, m_t, :],
                                     c1sb[:, m_t, 512:1024],
                                     start=False if n_t == 0 else (m_t == 0),
                                     stop=(m_t == NT - 1))
                nc.vector.tensor_copy(s1[:, n_t, 0:512], ps0[:])
                nc.vector.tensor_copy(s1[:, n_t, 512:1024], ps1[:])

            # pass 2: Zq_chunk = MM(S1, C2^T), 4 column strips of 512
            for l_s in range(4):
                pss = [psum_pool.tile([P, 512], F32, tag="ps",
                                      name=f"p2_{l_s}_{ks}")
                       for ks in range(8)]
                for n_t in range(NT):
                    c2t = c2_pool.tile([P, 512], F32R, tag="c2",
                                       name=f"c2_{l_s}_{n_t}")
                    nc.sync.dma_start(c2t[:], c2_p[l_s, n_t])
                    for ks in range(8):
                        nc.tensor.matmul(pss[ks][:],
                                         s1[:, n_t, P * ks:P * (ks + 1)],
                                         c2t[:],
                                         start=(n_t == 0),
                                         stop=(n_t == NT - 1))
                for ks in range(8):
                    ot = out_pool.tile([P, 512], F32, tag="out",
                                       name=f"o_{l_s}_{ks}")
                    nc.vector.tensor_copy(ot[:], pss[ks][:])
                    nc.sync.dma_start(
                        z[P * ks:P * (ks + 1), 512 * l_s:512 * (l_s + 1)],
                        ot[:])

    nc.compile()
    return nc


def _host_prep(x):
    """Fold x into the four quarter inputs and pack all DRAM operands."""
    x = np.asarray(x, dtype=np.float32)
    if "consts" not in _cache:
        ce, co = _half_dcts()
        c1c = {}  # (matrix, half) -> packed [P, NT, KCH]
        c2c = {}
        for nm, c in (("e", ce), ("o", co)):
            ct = np.ascontiguousarray(c.T)  # [m, k] float64
            for h in range(2):
                chunk = ct[:, KCH * h:KCH * (h + 1)]
                c1c[(nm, h)] = np.ascontiguousarray(
                    chunk.reshape(NT, P, KCH).transpose(1, 0, 2)
                ).astype(np.float32)
            c2c[nm] = np.ascontiguousarray(
                ct.reshape(NT, P, 4, 512).transpose(2, 0, 1, 3)
            ).astype(np.float32)
        _cache["consts"] = (c1c, c2c)
    c1c, c2c = _cache["consts"]

    xd = x.astype(np.float64)
    xtt = xd[:H, :H]
    xbt = xd[H:, :H][::-1, :]
    xtb = xd[:H, H:][:, ::-1]
    xbb = xd[H:, H:][::-1, ::-1]
    s_r = xtt + xbt        # row-fold sum
    d_r = xtt - xbt
    s_c = xtb + xbb        # row-fold of the col-mirrored half
    d_c = xtb - xbb
    quarters = {
        "ss": s_r + s_c,
        "ds": d_r + d_c,
        "sd": s_r - s_c,
        "dd": d_r - d_c,
    }

    def pack_a(a):
        return np.ascontiguousarray(
            a.reshape(NT, P, NT, P).transpose(2, 1, 0, 3)
        ).astype(np.float32)

    # quarter q -> (A, c1 matrix, c2 matrix, row parity, col parity)
    qdef = [("ss", "e", "e"), ("ds", "o", "e"),
            ("sd", "e", "o"), ("dd", "o", "o")]
    in_maps = []
    for core in range(NCORES):
        q, h = core // 2, core % 2
        aq, m1, m2 = qdef[q]
        in_maps.append({
            "a_p": pack_a(quarters[aq]),
            "c1_p": c1c[(m1, h)],
            "c2_p": c2c[m2],
        })
    return in_maps


def _run(x, trace=False):
    if "nc" not in _cache:
        _cache["nc"] = _build_nc()
    nc = _cache["nc"]
    in_maps = _host_prep(x)
    res = run_bass_kernel_spmd(nc, in_maps, list(range(NCORES)), trace=trace)

    z = np.empty((FULL, FULL), dtype=np.float32)
    pars = [(0, 0), (1, 0), (0, 1), (1, 1)]
    for core in range(NCORES):
        q, h = core // 2, core % 2
        rp, cp = pars[q]
        zq = res.results[core]["z"]
        z[2 * KCH * h + rp:2 * KCH * (h + 1) + rp:2, cp::2] = zq
    return z, res


def kernel(x):
    z, _ = _run(x, trace=False)
    return z


if __name__ == "__main__":
    rng = np.random.default_rng(0)
    x = rng.standard_normal((FULL, FULL), dtype=np.float32)
    z, res = _run(x, trace=os.environ.get("TRACE", "0") == "1")
    print("exec_time_ns:", res.exec_time_ns)


# revision 17
# speedup vs baseline: 1.0022x; 1.0022x over previous
"""2D DCT-II (unnormalized), 4096x4096, on 8 NeuronCores via Bass/Tile.

Math: Z = C @ X @ C^T with C[k,m] = cos(pi*k*(2m+1)/(2n)), n = 4096.

Even/odd folding on BOTH axes (C[k, n-1-m] = (-1)^k C[k, m]) splits the
transform into four independent half-size ones:

    Z[::2,  ::2] = Ce @ Ass @ Ce^T      Ass = Xtt + Xbt + Xtb + Xbb
    Z[1::2, ::2] = Co @ Ads @ Ce^T      Ads = Xtt - Xbt + Xtb - Xbb
    Z[::2, 1::2] = Ce @ Asd @ Co^T      Asd = Xtt + Xbt - Xtb - Xbb
    Z[1::2,1::2] = Co @ Add @ Co^T      Add = Xtt - Xbt - Xtb + Xbb

where Xtt = X[:h,:h], Xbt = X[h:,:h] row-mirrored, Xtb col-mirrored,
Xbb both, h = 2048, Ce/Co[r, m] = cos(pi*(2r|2r+1)*(2m+1)/(2n)).
The folds + final interleave run on host; the four 2048-transforms run on
the 8 cores (2 cores per quarter, each computing 1024 output rows).

On-device each core runs two matmul passes using the PE primitive
MM(A, B) = A^T @ B (contraction over partitions):

    S1 = MM(A, C1^T[:, chunk])     [2048, 1024]   (stays in SBUF)
    Zq = MM(S1, C2^T)              [1024, 2048]

No transposes, no cross-core communication. Matmuls run in float32r
(PE reads fp32 truncated to ~fp22; full rate for moving dim >= 256).
All DRAM operands are pre-packed on host so every DMA line is contiguous.
"""

import os
import numpy as np

import concourse.bacc as bacc
import concourse.mybir as mybir
import concourse.tile as tile
from concourse.bass_utils import run_bass_kernel_spmd

FULL = 4096
H = 2048                 # half size
P = 128                  # partitions
NCORES = 8
NT = H // P              # 16 tiles of 128 along a 2048 axis
KCH = 1024               # output rows per core (half of a quarter)
F32 = mybir.dt.float32
F32R = mybir.dt.float32r

_cache = {}


def _half_dcts():
    """Ce, Co as [r, m] (float64): rows 2r / 2r+1 of the full DCT matrix."""
    r = np.arange(H, dtype=np.float64)[:, None]
    m = np.arange(H, dtype=np.float64)[None, :]
    ce = np.cos(np.pi * (2 * r) * (2 * m + 1) / (2.0 * FULL))
    co = np.cos(np.pi * (2 * r + 1) * (2 * m + 1) / (2.0 * FULL))
    return ce, co


def _build_nc():
    nc = bacc.Bacc("TRN2", target_bir_lowering=False, debug=False,
                   num_devices=NCORES)
    # a_p[n_t, m_in, m_t, n_in] = A[128*m_t + m_in, 128*n_t + n_in]
    a_p = nc.dram_tensor("a_p", [NT, P, NT, P], F32R,
                         kind="ExternalInput").ap()
    # c1_p[m_in, m_t, k] = C1^T[128*m_t + m_in, KCH*h + k]
    c1_p = nc.dram_tensor("c1_p", [P, NT, KCH], F32R,
                          kind="ExternalInput").ap()
    # c2_p[l_s, n_t, n_in, l] = C2^T[128*n_t + n_in, 512*l_s + l]
    c2_p = nc.dram_tensor("c2_p", [4, NT, P, 512], F32R,
                          kind="ExternalInput").ap()
    z = nc.dram_tensor("z", [KCH, H], F32, kind="ExternalOutput").ap()

    with tile.TileContext(nc) as tc:
        with (
            tc.tile_pool(name="c1", bufs=1) as c1_pool,
            tc.tile_pool(name="s1p", bufs=1) as s1_pool,
            tc.tile_pool(name="ap", bufs=3) as a_pool,
            tc.tile_pool(name="c2", bufs=12) as c2_pool,
            tc.tile_pool(name="out", bufs=4) as out_pool,
            tc.tile_pool(name="ps", bufs=8, space="PSUM") as psum_pool,
        ):
            c1sb = c1_pool.tile([P, NT, KCH], F32R)
            s1 = s1_pool.tile([P, NT, KCH], F32R)

            # PE warmup: accumulate exact zeros into the first psum tiles
            # while the initial DMAs are in flight, so HAM reaches 2.4 GHz
            # before real work arrives (and the real m-loop starts with
            # start=False on pre-zeroed banks).
            zt = c1_pool.tile([P, 512], F32, name="zt")
            nc.gpsimd.memset(zt[:], 0.0)
            ztr = c1_pool.tile([P, 512], F32R, name="ztr")
            nc.vector.tensor_copy(ztr[:], zt[:])
            ps_first = psum_pool.tile([P, 512], F32, tag="ps", name="p1_0_0")
            NWARM = 36
            for w in range(NWARM):
                nc.tensor.matmul(ps_first[:], ztr[:, 0:P], ztr[:],
                                 start=(w == 0), stop=False)

            # pass 1 in two k'-strip sweeps: the first sweep only needs the
            # first 4 MiB half of c1, halving the head-critical DMA. The
            # a-stripes are re-read in sweep 1 (DMA has headroom over PE).
            for sw in range(2):
                lo, hi = 512 * sw, 512 * (sw + 1)
                for n_t in range(NT):
                    a_st = a_pool.tile([P, NT, P], F32R, tag="ap",
                                       name=f"a_{sw}_{n_t}")
                    for g in range(4):
                        nc.sync.dma_start(a_st[:, 4 * g:4 * (g + 1), :],
                                          a_p[n_t, :, 4 * g:4 * (g + 1), :])
                    if sw == 0 and n_t == 0:
                        ps = ps_first
                    else:
                        ps = psum_pool.tile([P, 512], F32, tag="ps",
                                            name=f"p1_{sw}_{n_t}")
                    for m_t in range(NT):
                        if sw == 0 and n_t == 0:
                            # strip-0 DMAs just-in-time, in consumption order
                            nc.sync.dma_start(c1sb[:, m_t, lo:hi],
                                              c1_p[:, m_t, lo:hi])
                        elif sw == 0 and n_t == 1:
                            # prefetch sweep-1's strips during sweep 0
                            nc.sync.dma_start(c1sb[:, m_t, 512:1024],
                                              c1_p[:, m_t, 512:1024])
                        nc.tensor.matmul(
                            ps[:], a_st[:, m_t, :], c1sb[:, m_t, lo:hi],
                            start=False if (sw == 0 and n_t == 0)
                            else (m_t == 0),
                            stop=(m_t == NT - 1))
                    nc.vector.tensor_copy(s1[:, n_t, lo:hi], ps[:])

            # pass 2: Zq_chunk = MM(S1, C2^T), 4 column strips of 512
            for l_s in range(4):
                pss = [psum_pool.tile([P, 512], F32, tag="ps",
                                      name=f"p2_{l_s}_{ks}")
                       for ks in range(8)]
                for n_t in range(NT):
                    c2t = c2_pool.tile([P, 512], F32R, tag="c2",
                                       name=f"c2_{l_s}_{n_t}")
                    nc.sync.dma_start(c2t[:], c2_p[l_s, n_t])
                    for ks in range(8):
                        nc.tensor.matmul(pss[ks][:],
                                         s1[:, n_t, P * ks:P * (ks + 1)],
                                         c2t[:],
                                         start=(n_t == 0),
                                         stop=(n_t == NT - 1))
                for ks in range(8):
                    ot = out_pool.tile([P, 512], F32, tag="out",
                                       name=f"o_{l_s}_{ks}")
                    nc.vector.tensor_copy(ot[:], pss[ks][:])
                    nc.sync.dma_start(
                        z[P * ks:P * (ks + 1), 512 * l_s:512 * (l_s + 1)],
                        ot[:])

    nc.compile()
    return nc


def _host_prep(x):
    """Fold x into the four quarter inputs and pack all DRAM operands."""
    x = np.asarray(x, dtype=np.float32)
    if "consts" not in _cache:
        ce, co = _half_dcts()
        c1c = {}  # (matrix, half) -> packed [P, NT, KCH]
        c2c = {}
        for nm, c in (("e", ce), ("o", co)):
            ct = np.ascontiguousarray(c.T)  # [m, k] float64
            for h in range(2):
                chunk = ct[:, KCH * h:KCH * (h + 1)]
                c1c[(nm, h)] = np.ascontiguousarray(
                    chunk.reshape(NT, P, KCH).transpose(1, 0, 2)
                ).astype(np.float32)
            c2c[nm] = np.ascontiguousarray(
                ct.reshape(NT, P, 4, 512).transpose(2, 0, 1, 3)
            ).astype(np.float32)
        _cache["consts"] = (c1c, c2c)
    c1c, c2c = _cache["consts"]

    xd = x.astype(np.float64)
    xtt = xd[:H, :H]
    xbt = xd[H:, :H][::-1, :]
    xtb = xd[:H, H:][:, ::-1]
    xbb = xd[H:, H:][::-1, ::-1]
    s_r = xtt + xbt        # row-fold sum
    d_r = xtt - xbt
    s_c = xtb + xbb        # row-fold of the col-mirrored half
    d_c = xtb - xbb
    quarters = {
        "ss": s_r + s_c,
        "ds": d_r + d_c,
        "sd": s_r - s_c,
        "dd": d_r - d_c,
    }

    def pack_a(a):
        return np.ascontiguousarray(
            a.reshape(NT, P, NT, P).transpose(2, 1, 0, 3)
        ).astype(np.float32)

    # quarter q -> (A, c1 matrix, c2 matrix, row parity, col parity)
    qdef = [("ss", "e", "e"), ("ds", "o", "e"),
            ("sd", "e", "o"), ("dd", "o", "o")]
    in_maps = []
    for core in range(NCORES):
        q, h = core // 2, core % 2
        aq, m1, m2 = qdef[q]
        in_maps.append({
            "a_p": pack_a(quarters[aq]),
            "c1_p": c1c[(m1, h)],
            "c2_p": c2c[m2],
        })
    return in_maps


def _run(x, trace=False):
    if "nc" not in _cache:
        _cache["nc"] = _build_nc()
    nc = _cache["nc"]
    in_maps = _host_prep(x)
    res = run_bass_kernel_spmd(nc, in_maps, list(range(NCORES)), trace=trace)

    z = np.empty((FULL, FULL), dtype=np.float32)
    pars = [(0, 0), (1, 0), (0, 1), (1, 1)]
    for core in range(NCORES):
        q, h = core // 2, core % 2
        rp, cp = pars[q]
        zq = res.results[core]["z"]
        z[2 * KCH * h + rp:2 * KCH * (h + 1) + rp:2, cp::2] = zq
    return z, res


def kernel(x):
    z, _ = _run(x, trace=False)
    return z


if __name__ == "__main__":
    rng = np.random.default_rng(0)
    x = rng.standard_normal((FULL, FULL), dtype=np.float32)
    z, res = _run(x, trace=os.environ.get("TRACE", "0") == "1")
    print("exec_time_ns:", res.exec_time_ns)


# revision 18
# speedup vs baseline: 1.0502x; 1.0479x over previous
"""2D DCT-II (unnormalized), 4096x4096, on 8 NeuronCores via Bass/Tile.

Math: Z = C @ X @ C^T with C[k,m] = cos(pi*k*(2m+1)/(2n)), n = 4096.

Even/odd folding on BOTH axes (C[k, n-1-m] = (-1)^k C[k, m]) splits the
transform into four independent half-size ones:

    Z[::2,  ::2] = Ce @ Ass @ Ce^T      Ass = Xtt + Xbt + Xtb + Xbb
    Z[1::2, ::2] = Co @ Ads @ Ce^T      Ads = Xtt - Xbt + Xtb - Xbb
    Z[::2, 1::2] = Ce @ Asd @ Co^T      Asd = Xtt + Xbt - Xtb - Xbb
    Z[1::2,1::2] = Co @ Add @ Co^T      Add = Xtt - Xbt - Xtb + Xbb

where Xtt = X[:h,:h], Xbt = X[h:,:h] row-mirrored, Xtb col-mirrored,
Xbb both, h = 2048, Ce/Co[r, m] = cos(pi*(2r|2r+1)*(2m+1)/(2n)).
The folds + final interleave run on host; the four 2048-transforms run on
the 8 cores (2 cores per quarter, each computing 1024 output rows).

On-device each core runs two matmul passes using the PE primitive
MM(A, B) = A^T @ B (contraction over partitions):

    S1 = MM(A, C1^T[:, chunk])     [2048, 1024]   (stays in SBUF)
    Zq = MM(S1, C2^T)              [1024, 2048]

No transposes, no cross-core communication. Matmuls run in float32r
(PE reads fp32 truncated to ~fp22; full rate for moving dim >= 256).
All DRAM operands are pre-packed on host so every DMA line is contiguous.
"""

import os
import numpy as np

import concourse.bacc as bacc
import concourse.mybir as mybir
import concourse.tile as tile
from concourse.bass_utils import run_bass_kernel_spmd

FULL = 4096
H = 2048                 # half size
P = 128                  # partitions
NCORES = 8
NT = H // P              # 16 tiles of 128 along a 2048 axis
KCH = 1024               # output rows per core (half of a quarter)
F32 = mybir.dt.float32
F32R = mybir.dt.float32r

_cache = {}


def _half_dcts():
    """Ce, Co as [r, m] (float64): rows 2r / 2r+1 of the full DCT matrix."""
    r = np.arange(H, dtype=np.float64)[:, None]
    m = np.arange(H, dtype=np.float64)[None, :]
    ce = np.cos(np.pi * (2 * r) * (2 * m + 1) / (2.0 * FULL))
    co = np.cos(np.pi * (2 * r + 1) * (2 * m + 1) / (2.0 * FULL))
    return ce, co


def _build_nc():
    nc = bacc.Bacc("TRN2", target_bir_lowering=False, debug=False,
                   num_devices=NCORES)
    # a_p[n_t, m_in, m_t, n_in] = A[128*m_t + m_in, 128*n_t + n_in]
    a_p = nc.dram_tensor("a_p", [NT, P, NT, P], F32R,
                         kind="ExternalInput").ap()
    # c1_p[m_in, m_t, k] = C1^T[128*m_t + m_in, KCH*h + k]
    c1_p = nc.dram_tensor("c1_p", [P, NT, KCH], F32R,
                          kind="ExternalInput").ap()
    # c2_p[l_s, n_t, n_in, l] = C2^T[128*n_t + n_in, 512*l_s + l]
    c2_p = nc.dram_tensor("c2_p", [4, NT, P, 512], F32R,
                          kind="ExternalInput").ap()
    z = nc.dram_tensor("z", [KCH, H], F32, kind="ExternalOutput").ap()

    with tile.TileContext(nc) as tc:
        with (
            tc.tile_pool(name="c1", bufs=1) as c1_pool,
            tc.tile_pool(name="s1p", bufs=1) as s1_pool,
            tc.tile_pool(name="ap", bufs=3) as a_pool,
            tc.tile_pool(name="c2", bufs=12) as c2_pool,
            tc.tile_pool(name="out", bufs=4) as out_pool,
            tc.tile_pool(name="ps", bufs=8, space="PSUM") as psum_pool,
        ):
            c1sb = c1_pool.tile([P, NT, KCH], F32R)
            s1 = s1_pool.tile([P, NT, KCH], F32R)

            # PE warmup: accumulate exact zeros into the first psum tiles
            # while the initial DMAs are in flight, so HAM reaches 2.4 GHz
            # before real work arrives (and the real m-loop starts with
            # start=False on pre-zeroed banks).
            zt = c1_pool.tile([P, 512], F32, name="zt")
            nc.gpsimd.memset(zt[:], 0.0)
            ztr = c1_pool.tile([P, 512], F32R, name="ztr")
            nc.vector.tensor_copy(ztr[:], zt[:])
            ps0_first = psum_pool.tile([P, 512], F32, tag="ps", name="p1a_0")
            ps1_first = psum_pool.tile([P, 512], F32, tag="ps", name="p1b_0")
            NWARM = 36
            for w in range(NWARM):
                tgt = ps0_first if w % 2 == 0 else ps1_first
                nc.tensor.matmul(tgt[:], ztr[:, 0:P], ztr[:],
                                 start=(w < 2), stop=False)

            # pass 1: S1[:, n_t, :] = sum_m A[m, n_t-block]^T @ C1^T-chunk
            # n_t == 0 interleaves the c1 strip loads in consumption order.
            for n_t in range(NT):
                a_st = a_pool.tile([P, NT, P], F32R, tag="ap",
                                   name=f"a_{n_t}")
                for g in range(4):
                    nc.sync.dma_start(a_st[:, 4 * g:4 * (g + 1), :],
                                      a_p[n_t, :, 4 * g:4 * (g + 1), :])
                if n_t == 0:
                    ps0, ps1 = ps0_first, ps1_first
                else:
                    ps0 = psum_pool.tile([P, 512], F32, tag="ps",
                                         name=f"p1a_{n_t}")
                    ps1 = psum_pool.tile([P, 512], F32, tag="ps",
                                         name=f"p1b_{n_t}")
                for m_t in range(NT):
                    if n_t == 0:
                        for s in range(2):
                            nc.sync.dma_start(
                                c1sb[:, m_t, 512 * s:512 * (s + 1)],
                                c1_p[:, m_t, 512 * s:512 * (s + 1)])
                    nc.tensor.matmul(ps0[:], a_st[:, m_t, :],
                                     c1sb[:, m_t, 0:512],
                                     start=False if n_t == 0 else (m_t == 0),
                                     stop=(m_t == NT - 1))
                    nc.tensor.matmul(ps1[:], a_st[:, m_t, :],
                                     c1sb[:, m_t, 512:1024],
                                     start=False if n_t == 0 else (m_t == 0),
                                     stop=(m_t == NT - 1))
                nc.vector.tensor_copy(s1[:, n_t, 0:512], ps0[:])
                nc.vector.tensor_copy(s1[:, n_t, 512:1024], ps1[:])

            # pass 2: Zq_chunk = MM(S1, C2^T), 4 column strips of 512
            for l_s in range(4):
                pss = [psum_pool.tile([P, 512], F32, tag="ps",
                                      name=f"p2_{l_s}_{ks}")
                       for ks in range(8)]
                for n_t in range(NT):
                    c2t = c2_pool.tile([P, 512], F32R, tag="c2",
                                       name=f"c2_{l_s}_{n_t}")
                    nc.sync.dma_start(c2t[:], c2_p[l_s, n_t])
                    for ks in range(8):
                        nc.tensor.matmul(pss[ks][:],
                                         s1[:, n_t, P * ks:P * (ks + 1)],
                                         c2t[:],
                                         start=(n_t == 0),
                                         stop=(n_t == NT - 1))
                for ks in range(8):
                    ot = out_pool.tile([P, 512], F32, tag="out",
                                       name=f"o_{l_s}_{ks}")
                    nc.vector.tensor_copy(ot[:], pss[ks][:])
                    nc.sync.dma_start(
                        z[P * ks:P * (ks + 1), 512 * l_s:512 * (l_s + 1)],
                        ot[:])

    nc.compile()
    return nc


def _host_prep(x):
    """Fold x into the four quarter inputs and pack all DRAM operands."""
    x = np.asarray(x, dtype=np.float32)
    if "consts" not in _cache:
        ce, co = _half_dcts()
        c1c = {}  # (matrix, half) -> packed [P, NT, KCH]
        c2c = {}
        for nm, c in (("e", ce), ("o", co)):
            ct = np.ascontiguousarray(c.T)  # [m, k] float64
            for h in range(2):
                chunk = ct[:, KCH * h:KCH * (h + 1)]
                c1c[(nm, h)] = np.ascontiguousarray(
                    chunk.reshape(NT, P, KCH).transpose(1, 0, 2)
                ).astype(np.float32)
            c2c[nm] = np.ascontiguousarray(
                ct.reshape(NT, P, 4, 512).transpose(2, 0, 1, 3)
            ).astype(np.float32)
        _cache["consts"] = (c1c, c2c)
    c1c, c2c = _cache["consts"]

    xd = x.astype(np.float64)
    xtt = xd[:H, :H]
    xbt = xd[H:, :H][::-1, :]
    xtb = xd[:H, H:][:, ::-1]
    xbb = xd[H:, H:][::-1, ::-1]
    s_r = xtt + xbt        # row-fold sum
    d_r = xtt - xbt
    s_c = xtb + xbb        # row-fold of the col-mirrored half
    d_c = xtb - xbb
    quarters = {
        "ss": s_r + s_c,
        "ds": d_r + d_c,
        "sd": s_r - s_c,
        "dd": d_r - d_c,
    }

    def pack_a(a):
        return np.ascontiguousarray(
            a.reshape(NT, P, NT, P).transpose(2, 1, 0, 3)
        ).astype(np.float32)

    # quarter q -> (A, c1 matrix, c2 matrix, row parity, col parity)
    qdef = [("ss", "e", "e"), ("ds", "o", "e"),
            ("sd", "e", "o"), ("dd", "o", "o")]
    in_maps = []
    for core in range(NCORES):
        q, h = core // 2, core % 2
        aq, m1, m2 = qdef[q]
        in_maps.append({
            "a_p": pack_a(quarters[aq]),
            "c1_p": c1c[(m1, h)],
            "c2_p": c2c[m2],
        })
    return in_maps


def _run(x, trace=False):
    if "nc" not in _cache:
        _cache["nc"] = _build_nc()
    nc = _cache["nc"]
    in_maps = _host_prep(x)
    res = run_bass_kernel_spmd(nc, in_maps, list(range(NCORES)), trace=trace)

    z = np.empty((FULL, FULL), dtype=np.float32)
    pars = [(0, 0), (1, 0), (0, 1), (1, 1)]
    for core in range(NCORES):
        q, h = core // 2, core % 2
        rp, cp = pars[q]
        zq = res.results[core]["z"]
        z[2 * KCH * h + rp:2 * KCH * (h + 1) + rp:2, cp::2] = zq
    return z, res


def kernel(x):
    z, _ = _run(x, trace=False)
    return z


if __name__ == "__main__":
    rng = np.random.default_rng(0)
    x = rng.standard_normal((FULL, FULL), dtype=np.float32)
    z, res = _run(x, trace=os.environ.get("TRACE", "0") == "1")
    print("exec_time_ns:", res.exec_time_ns)


# revision 20
# speedup vs baseline: 1.0721x; 1.0209x over previous
"""2D DCT-II (unnormalized), 4096x4096, on 8 NeuronCores via Bass/Tile.

Math: Z = C @ X @ C^T with C[k,m] = cos(pi*k*(2m+1)/(2n)), n = 4096.

Even/odd folding on BOTH axes (C[k, n-1-m] = (-1)^k C[k, m]) splits the
transform into four independent half-size ones:

    Z[::2,  ::2] = Ce @ Ass @ Ce^T      Ass = Xtt + Xbt + Xtb + Xbb
    Z[1::2, ::2] = Co @ Ads @ Ce^T      Ads = Xtt - Xbt + Xtb - Xbb
    Z[::2, 1::2] = Ce @ Asd @ Co^T      Asd = Xtt + Xbt - Xtb - Xbb
    Z[1::2,1::2] = Co @ Add @ Co^T      Add = Xtt - Xbt - Xtb + Xbb

where Xtt = X[:h,:h], Xbt = X[h:,:h] row-mirrored, Xtb col-mirrored,
Xbb both, h = 2048, Ce/Co[r, m] = cos(pi*(2r|2r+1)*(2m+1)/(2n)).
The folds + final interleave run on host; the four 2048-transforms run on
the 8 cores (2 cores per quarter, each computing 1024 output rows).

On-device each core runs two matmul passes using the PE primitive
MM(A, B) = A^T @ B (contraction over partitions):

    S1 = MM(A, C1^T[:, chunk])     [2048, 1024]   (stays in SBUF)
    Zq = MM(S1, C2^T)              [1024, 2048]

No transposes, no cross-core communication. Matmuls run in float32r
(PE reads fp32 truncated to ~fp22; full rate for moving dim >= 256).
All DRAM operands are pre-packed on host so every DMA line is contiguous.
"""

import os
import numpy as np

import concourse.bacc as bacc
import concourse.mybir as mybir
import concourse.tile as tile
from concourse.bass_utils import run_bass_kernel_spmd

FULL = 4096
H = 2048                 # half size
P = 128                  # partitions
NCORES = 8
NT = H // P              # 16 tiles of 128 along a 2048 axis
KCH = 1024               # output rows per core (half of a quarter)
F32 = mybir.dt.float32
F32R = mybir.dt.float32r

_cache = {}


def _half_dcts():
    """Ce, Co as [r, m] (float64): rows 2r / 2r+1 of the full DCT matrix."""
    r = np.arange(H, dtype=np.float64)[:, None]
    m = np.arange(H, dtype=np.float64)[None, :]
    ce = np.cos(np.pi * (2 * r) * (2 * m + 1) / (2.0 * FULL))
    co = np.cos(np.pi * (2 * r + 1) * (2 * m + 1) / (2.0 * FULL))
    return ce, co


def _build_nc():
    nc = bacc.Bacc("TRN2", target_bir_lowering=False, debug=False,
                   num_devices=NCORES)
    # a_p[n_t, m_in, m_t, n_in] = A[128*m_t + m_in, 128*n_t + n_in]
    a_p = nc.dram_tensor("a_p", [NT, P, NT, P], F32R,
                         kind="ExternalInput").ap()
    # c1_p[m_in, m_t, k] = C1^T[128*m_t + m_in, KCH*h + k]
    c1_p = nc.dram_tensor("c1_p", [P, NT, KCH], F32R,
                          kind="ExternalInput").ap()
    # c2_p[l_s, n_t, n_in, l] = C2^T[128*n_t + n_in, 512*l_s + l]
    c2_p = nc.dram_tensor("c2_p", [4, NT, P, 512], F32R,
                          kind="ExternalInput").ap()
    z = nc.dram_tensor("z", [KCH, H], F32, kind="ExternalOutput").ap()

    with tile.TileContext(nc) as tc:
        with (
            tc.tile_pool(name="c1", bufs=1) as c1_pool,
            tc.tile_pool(name="s1p", bufs=1) as s1_pool,
            tc.tile_pool(name="ap", bufs=4) as a_pool,
            tc.tile_pool(name="c2", bufs=8) as c2_pool,
            tc.tile_pool(name="out", bufs=4) as out_pool,
            tc.tile_pool(name="ps", bufs=8, space="PSUM") as psum_pool,
        ):
            c1sb = c1_pool.tile([P, NT, KCH], F32R)
            s1 = s1_pool.tile([P, NT, KCH], F32R)

            # PE warmup: accumulate exact zeros into the first psum tiles
            # while the initial DMAs are in flight, so HAM reaches 2.4 GHz
            # before real work arrives (and the real m-loop starts with
            # start=False on pre-zeroed banks).
            zt = c1_pool.tile([P, 512], F32, name="zt")
            nc.gpsimd.memset(zt[:], 0.0)
            ztr = c1_pool.tile([P, 512], F32R, name="ztr")
            nc.vector.tensor_copy(ztr[:], zt[:])
            ps0_first = psum_pool.tile([P, 512], F32, tag="ps", name="p1a_0")
            ps1_first = psum_pool.tile([P, 512], F32, tag="ps", name="p1b_0")
            NWARM = 36
            for w in range(NWARM):
                tgt = ps0_first if w % 2 == 0 else ps1_first
                nc.tensor.matmul(tgt[:], ztr[:, 0:P], ztr[:],
                                 start=(w < 2), stop=False)

            # pass 1: S1[:, n_t, :] = sum_m A[m, n_t-block]^T @ C1^T-chunk
            # n_t == 0 interleaves the c1 strip loads in consumption order.
            for n_t in range(NT):
                a_st = a_pool.tile([P, NT, P], F32R, tag="ap",
                                   name=f"a_{n_t}")
                for g in range(4):
                    nc.sync.dma_start(a_st[:, 4 * g:4 * (g + 1), :],
                                      a_p[n_t, :, 4 * g:4 * (g + 1), :])
                if n_t == 0:
                    ps0, ps1 = ps0_first, ps1_first
                else:
                    ps0 = psum_pool.tile([P, 512], F32, tag="ps",
                                         name=f"p1a_{n_t}")
                    ps1 = psum_pool.tile([P, 512], F32, tag="ps",
                                         name=f"p1b_{n_t}")
                for m_t in range(NT):
                    if n_t == 0:
                        for s in range(2):
                            nc.sync.dma_start(
                                c1sb[:, m_t, 512 * s:512 * (s + 1)],
                                c1_p[:, m_t, 512 * s:512 * (s + 1)])
                    nc.tensor.matmul(ps0[:], a_st[:, m_t, :],
                                     c1sb[:, m_t, 0:512],
                                     start=False if n_t == 0 else (m_t == 0),
                                     stop=(m_t == NT - 1))
                    nc.tensor.matmul(ps1[:], a_st[:, m_t, :],
                                     c1sb[:, m_t, 512:1024],
                                     start=False if n_t == 0 else (m_t == 0),
                                     stop=(m_t == NT - 1))
                nc.vector.tensor_copy(s1[:, n_t, 0:512], ps0[:])
                nc.vector.tensor_copy(s1[:, n_t, 512:1024], ps1[:])

            # pass 2: Zq_chunk = MM(S1, C2^T), 4 column strips of 512
            for l_s in range(4):
                pss = [psum_pool.tile([P, 512], F32, tag="ps",
                                      name=f"p2_{l_s}_{ks}")
                       for ks in range(8)]
                for n_t in range(NT):
                    c2t = c2_pool.tile([P, 512], F32R, tag="c2",
                                       name=f"c2_{l_s}_{n_t}")
                    nc.sync.dma_start(c2t[:], c2_p[l_s, n_t])
                    for ks in range(8):
                        nc.tensor.matmul(pss[ks][:],
                                         s1[:, n_t, P * ks:P * (ks + 1)],
                                         c2t[:],
                                         start=(n_t == 0),
                                         stop=(n_t == NT - 1))
                for ks in range(8):
                    ot = out_pool.tile([P, 512], F32, tag="out",
                                       name=f"o_{l_s}_{ks}")
                    nc.vector.tensor_copy(ot[:], pss[ks][:])
                    nc.sync.dma_start(
                        z[P * ks:P * (ks + 1), 512 * l_s:512 * (l_s + 1)],
                        ot[:])

    nc.compile()
    return nc


def _host_prep(x):
    """Fold x into the four quarter inputs and pack all DRAM operands."""
    x = np.asarray(x, dtype=np.float32)
    if "consts" not in _cache:
        ce, co = _half_dcts()
        c1c = {}  # (matrix, half) -> packed [P, NT, KCH]
        c2c = {}
        for nm, c in (("e", ce), ("o", co)):
            ct = np.ascontiguousarray(c.T)  # [m, k] float64
            for h in range(2):
                chunk = ct[:, KCH * h:KCH * (h + 1)]
                c1c[(nm, h)] = np.ascontiguousarray(
                    chunk.reshape(NT, P, KCH).transpose(1, 0, 2)
                ).astype(np.float32)
            c2c[nm] = np.ascontiguousarray(
                ct.reshape(NT, P, 4, 512).transpose(2, 0, 1, 3)
            ).astype(np.float32)
        _cache["consts"] = (c1c, c2c)
    c1c, c2c = _cache["consts"]

    xd = x.astype(np.float64)
    xtt = xd[:H, :H]
    xbt = xd[H:, :H][::-1, :]
    xtb = xd[:H, H:][:, ::-1]
    xbb = xd[H:, H:][::-1, ::-1]
    s_r = xtt + xbt        # row-fold sum
    d_r = xtt - xbt
    s_c = xtb + xbb        # row-fold of the col-mirrored half
    d_c = xtb - xbb
    quarters = {
        "ss": s_r + s_c,
        "ds": d_r + d_c,
        "sd": s_r - s_c,
        "dd": d_r - d_c,
    }

    def pack_a(a):
        return np.ascontiguousarray(
            a.reshape(NT, P, NT, P).transpose(2, 1, 0, 3)
        ).astype(np.float32)

    # quarter q -> (A, c1 matrix, c2 matrix, row parity, col parity)
    qdef = [("ss", "e", "e"), ("ds", "o", "e"),
            ("sd", "e", "o"), ("dd", "o", "o")]
    in_maps = []
    for core in range(NCORES):
        q, h = core // 2, core % 2
        aq, m1, m2 = qdef[q]
        in_maps.append({
            "a_p": pack_a(quarters[aq]),
            "c1_p": c1c[(m1, h)],
            "c2_p": c2c[m2],
        })
    return in_maps


def _run(x, trace=False):
    if "nc" not in _cache:
        _cache["nc"] = _build_nc()
    nc = _cache["nc"]
    in_maps = _host_prep(x)
    res = None
    last_err = None
    for attempt in range(3):
        try:
            res = run_bass_kernel_spmd(nc, in_maps, list(range(NCORES)),
                                       trace=trace)
            break
        except Exception as e:  # transient NRT device errors happen
            last_err = e
            import time
            time.sleep(3.0)
    if res is None:
        raise last_err

    z = np.empty((FULL, FULL), dtype=np.float32)
    pars = [(0, 0), (1, 0), (0, 1), (1, 1)]
    for core in range(NCORES):
        q, h = core // 2, core % 2
        rp, cp = pars[q]
        zq = res.results[core]["z"]
        z[2 * KCH * h + rp:2 * KCH * (h + 1) + rp:2, cp::2] = zq
    return z, res


def kernel(x):
    z, _ = _run(x, trace=False)
    return z


if __name__ == "__main__":
    rng = np.random.default_rng(0)
    x = rng.standard_normal((FULL, FULL), dtype=np.float32)
    z, res = _run(x, trace=os.environ.get("TRACE", "0") == "1")
    print("exec_time_ns:", res.exec_time_ns)


# revision 25
# speedup vs baseline: 1.0886x; 1.0154x over previous
"""2D DCT-II (unnormalized), 4096x4096, on 8 NeuronCores via Bass/Tile.

Math: Z = C @ X @ C^T with C[k,m] = cos(pi*k*(2m+1)/(2n)), n = 4096.

Even/odd folding on BOTH axes (C[k, n-1-m] = (-1)^k C[k, m]) splits the
transform into four independent half-size ones:

    Z[::2,  ::2] = Ce @ Ass @ Ce^T      Ass = Xtt + Xbt + Xtb + Xbb
    Z[1::2, ::2] = Co @ Ads @ Ce^T      Ads = Xtt - Xbt + Xtb - Xbb
    Z[::2, 1::2] = Ce @ Asd @ Co^T      Asd = Xtt + Xbt - Xtb - Xbb
    Z[1::2,1::2] = Co @ Add @ Co^T      Add = Xtt - Xbt - Xtb + Xbb

where Xtt = X[:h,:h], Xbt = X[h:,:h] row-mirrored, Xtb col-mirrored,
Xbb both, h = 2048, Ce/Co[r, m] = cos(pi*(2r|2r+1)*(2m+1)/(2n)).
The folds + final interleave run on host; the four 2048-transforms run on
the 8 cores (2 cores per quarter, each computing 1024 output rows).

On-device each core runs two matmul passes using the PE primitive
MM(A, B) = A^T @ B (contraction over partitions):

    S1 = MM(A, C1^T[:, chunk])     [2048, 1024]   (stays in SBUF)
    Zq = MM(S1, C2^T)              [1024, 2048]

No transposes, no cross-core communication. Matmuls run in float32r
(PE reads fp32 truncated to ~fp22; full rate for moving dim >= 256).
All DRAM operands are pre-packed on host so every DMA line is contiguous.
"""

import os
import numpy as np

import concourse.bacc as bacc
import concourse.mybir as mybir
import concourse.tile as tile
from concourse.bass_utils import run_bass_kernel_spmd

FULL = 4096
H = 2048                 # half size
P = 128                  # partitions
NCORES = 8
NT = H // P              # 16 tiles of 128 along a 2048 axis
KCH = 1024               # output rows per core (half of a quarter)
F32 = mybir.dt.float32
F32R = mybir.dt.float32r

_cache = {}


def _half_dcts():
    """Ce, Co as [r, m] (float64): rows 2r / 2r+1 of the full DCT matrix."""
    r = np.arange(H, dtype=np.float64)[:, None]
    m = np.arange(H, dtype=np.float64)[None, :]
    ce = np.cos(np.pi * (2 * r) * (2 * m + 1) / (2.0 * FULL))
    co = np.cos(np.pi * (2 * r + 1) * (2 * m + 1) / (2.0 * FULL))
    return ce, co


def _build_nc():
    nc = bacc.Bacc("TRN2", target_bir_lowering=False, debug=False,
                   num_devices=NCORES)
    # a_p[n_t, m_in, m_t, n_in] = A[128*m_t + m_in, 128*n_t + n_in]
    a_p = nc.dram_tensor("a_p", [NT, P, NT, P], F32R,
                         kind="ExternalInput").ap()
    # c1_p[m_in, m_t, k] = C1^T[128*m_t + m_in, KCH*h + k]
    c1_p = nc.dram_tensor("c1_p", [P, NT, KCH], F32R,
                          kind="ExternalInput").ap()
    # c2_p[l_c, n_in, n_t, l_in] = C2^T[128*n_t + n_in, 128*l_c + l_in]
    c2_p = nc.dram_tensor("c2_p", [NT, P, NT, P], F32R,
                          kind="ExternalInput").ap()
    # z holds Zq^T: z[l, k'] (host transposes back)
    z = nc.dram_tensor("z", [H, KCH], F32, kind="ExternalOutput").ap()

    with tile.TileContext(nc) as tc:
        with (
            tc.tile_pool(name="c1", bufs=1) as c1_pool,
            tc.tile_pool(name="s1p", bufs=1) as s1_pool,
            tc.tile_pool(name="ap", bufs=3) as a_pool,
            tc.tile_pool(name="c2", bufs=2) as c2_pool,
            tc.tile_pool(name="out", bufs=4) as out_pool,
            tc.tile_pool(name="ps", bufs=8, space="PSUM") as psum_pool,
        ):
            c1sb = c1_pool.tile([P, NT, KCH], F32R)
            s1 = s1_pool.tile([P, NT, KCH], F32R)

            # PE warmup: accumulate exact zeros into the first psum tiles
            # while the initial DMAs are in flight, so HAM reaches 2.4 GHz
            # before real work arrives (and the real m-loop starts with
            # start=False on pre-zeroed banks).
            zt = c1_pool.tile([P, 512], F32, name="zt")
            nc.gpsimd.memset(zt[:], 0.0)
            ztr = c1_pool.tile([P, 512], F32R, name="ztr")
            nc.vector.tensor_copy(ztr[:], zt[:])
            ps0_first = psum_pool.tile([P, 512], F32, tag="ps", name="p1a_0")
            ps1_first = psum_pool.tile([P, 512], F32, tag="ps", name="p1b_0")
            NWARM = 36
            for w in range(NWARM):
                tgt = ps0_first if w % 2 == 0 else ps1_first
                nc.tensor.matmul(tgt[:], ztr[:, 0:P], ztr[:],
                                 start=(w < 2), stop=False)

            # pass 1: S1[:, n_t, :] = sum_m A[m, n_t-block]^T @ C1^T-chunk
            # n_t == 0 interleaves the c1 strip loads in consumption order.
            for n_t in range(NT):
                a_st = a_pool.tile([P, NT, P], F32R, tag="ap",
                                   name=f"a_{n_t}")
                for g in range(4):
                    nc.sync.dma_start(a_st[:, 4 * g:4 * (g + 1), :],
                                      a_p[n_t, :, 4 * g:4 * (g + 1), :])
                if n_t == 0:
                    ps0, ps1 = ps0_first, ps1_first
                else:
                    ps0 = psum_pool.tile([P, 512], F32, tag="ps",
                                         name=f"p1a_{n_t}")
                    ps1 = psum_pool.tile([P, 512], F32, tag="ps",
                                         name=f"p1b_{n_t}")
                for m_t in range(NT):
                    if n_t == 0:
                        for s in range(2):
                            nc.sync.dma_start(
                                c1sb[:, m_t, 512 * s:512 * (s + 1)],
                                c1_p[:, m_t, 512 * s:512 * (s + 1)])
                    nc.tensor.matmul(ps0[:], a_st[:, m_t, :],
                                     c1sb[:, m_t, 0:512],
                                     start=False if n_t == 0 else (m_t == 0),
                                     stop=(m_t == NT - 1))
                    nc.tensor.matmul(ps1[:], a_st[:, m_t, :],
                                     c1sb[:, m_t, 512:1024],
                                     start=False if n_t == 0 else (m_t == 0),
                                     stop=(m_t == NT - 1))
                nc.vector.tensor_copy(s1[:, n_t, 0:512], ps0[:])
                nc.vector.tensor_copy(s1[:, n_t, 512:1024], ps1[:])

            # pass 2 (Z^T orientation): out[l, k'] = MM(c2-tile, s1-strip).
            # The stationary c2 tile is reused for both k'-strips, halving
            # weight loads; only 2 psum banks are live at a time.
            for l_c in range(NT):
                c2st = c2_pool.tile([P, NT, P], F32R, tag="c2",
                                    name=f"c2_{l_c}")
                for g in range(4):
                    nc.sync.dma_start(c2st[:, 4 * g:4 * (g + 1), :],
                                      c2_p[l_c, :, 4 * g:4 * (g + 1), :])
                psa = psum_pool.tile([P, 512], F32, tag="ps",
                                     name=f"p2a_{l_c}")
                psb = psum_pool.tile([P, 512], F32, tag="ps",
                                     name=f"p2b_{l_c}")
                for n_t in range(NT):
                    nc.tensor.matmul(psa[:], c2st[:, n_t, :],
                                     s1[:, n_t, 0:512],
                                     start=(n_t == 0), stop=(n_t == NT - 1))
                    nc.tensor.matmul(psb[:], c2st[:, n_t, :],
                                     s1[:, n_t, 512:1024],
                                     start=(n_t == 0), stop=(n_t == NT - 1))
                for s, ps in ((0, psa), (1, psb)):
                    ot = out_pool.tile([P, 512], F32, tag="out",
                                       name=f"o_{l_c}_{s}")
                    nc.vector.tensor_copy(ot[:], ps[:])
                    nc.sync.dma_start(
                        z[P * l_c:P * (l_c + 1), 512 * s:512 * (s + 1)],
                        ot[:])

    nc.compile()
    return nc


def _host_prep(x):
    """Fold x into the four quarter inputs and pack all DRAM operands."""
    x = np.asarray(x, dtype=np.float32)
    if "consts" not in _cache:
        ce, co = _half_dcts()
        c1c = {}  # (matrix, half) -> packed [P, NT, KCH]
        c2c = {}
        for nm, c in (("e", ce), ("o", co)):
            ct = np.ascontiguousarray(c.T)  # [m, k] float64
            for h in range(2):
                chunk = ct[:, KCH * h:KCH * (h + 1)]
                c1c[(nm, h)] = np.ascontiguousarray(
                    chunk.reshape(NT, P, KCH).transpose(1, 0, 2)
                ).astype(np.float32)
            c2c[nm] = np.ascontiguousarray(
                ct.reshape(NT, P, NT, P).transpose(2, 1, 0, 3)
            ).astype(np.float32)
        _cache["consts"] = (c1c, c2c)
    c1c, c2c = _cache["consts"]

    xd = x.astype(np.float64)
    xtt = xd[:H, :H]
    xbt = xd[H:, :H][::-1, :]
    xtb = xd[:H, H:][:, ::-1]
    xbb = xd[H:, H:][::-1, ::-1]
    s_r = xtt + xbt        # row-fold sum
    d_r = xtt - xbt
    s_c = xtb + xbb        # row-fold of the col-mirrored half
    d_c = xtb - xbb
    quarters = {
        "ss": s_r + s_c,
        "ds": d_r + d_c,
        "sd": s_r - s_c,
        "dd": d_r - d_c,
    }

    def pack_a(a):
        return np.ascontiguousarray(
            a.reshape(NT, P, NT, P).transpose(2, 1, 0, 3)
        ).astype(np.float32)

    # quarter q -> (A, c1 matrix, c2 matrix, row parity, col parity)
    qdef = [("ss", "e", "e"), ("ds", "o", "e"),
            ("sd", "e", "o"), ("dd", "o", "o")]
    in_maps = []
    for core in range(NCORES):
        q, h = core // 2, core % 2
        aq, m1, m2 = qdef[q]
        in_maps.append({
            "a_p": pack_a(quarters[aq]),
            "c1_p": c1c[(m1, h)],
            "c2_p": c2c[m2],
        })
    return in_maps


def _run(x, trace=False):
    if "nc" not in _cache:
        _cache["nc"] = _build_nc()
    nc = _cache["nc"]
    in_maps = _host_prep(x)
    res = None
    last_err = None
    for attempt in range(3):
        try:
            res = run_bass_kernel_spmd(nc, in_maps, list(range(NCORES)),
                                       trace=trace)
            break
        except Exception as e:  # transient NRT device errors happen
            last_err = e
            import time
            time.sleep(3.0)
    if res is None:
        raise last_err

    z = np.empty((FULL, FULL), dtype=np.float32)
    pars = [(0, 0), (1, 0), (0, 1), (1, 1)]
    for core in range(NCORES):
        q, h = core // 2, core % 2
        rp, cp = pars[q]
        zq = res.results[core]["z"].T  # device wrote Zq^T
        z[2 * KCH * h + rp:2 * KCH * (h + 1) + rp:2, cp::2] = zq
    return z, res


def kernel(x):
    z, _ = _run(x, trace=False)
    return z


if __name__ == "__main__":
    rng = np.random.default_rng(0)
    x = rng.standard_normal((FULL, FULL), dtype=np.float32)
    z, res = _run(x, trace=os.environ.get("TRACE", "0") == "1")
    print("exec_time_ns:", res.exec_time_ns)
